# revision 14
# baseline (speedup 1.0000x reference)
"""Focal + GIoU criterion on 8 Trainium2 NeuronCores.

Data-parallel over B=8 (one batch row per core). Each core computes three
partial scalars (valid-masked focal sum, fg-masked (1-giou) sum, fg count);
the host combines them and applies the shared num_fg normalization, mirroring
the all-reduce of num_foreground in the reference.

Per-core layout: anchor a = t*4096 + p*32 + k  (t: tile 0..15, p: partition
0..127, k: slot 0..31). pred_cls tiles are [128, 32*80] f32.

Math (exact, single ACT table set natural_log_exp_and_others):
  e2 = exp(x); w = ln(1+e2) = softplus(x); sq = exp(2(x-w)) = sigmoid(x)^2
  background focal = 0.75 * w * sq, summed with the valid mask (STT accum).
  target-class correction per fg anchor, from the gathered logit x_t:
    corr = 0.25*softplus(-x_t)*(1-s)^2 - 0.75*softplus(x_t)*s^2
  gathered via gpsimd indirect_copy (indices shared per 16-partition group,
  so each group gathers all 16 lanes' targets and a constant select mask
  picks the diagonal).

GIoU emission is interleaved into the tile loop (it only depends on the box
DMAs) and the correction runs in two column batches so the kernel tail stays
short.
"""
import sys
import numpy as np

for _p in ("/opt/trn_rl_repo", "/root/.axon_site/_ro/trn_rl_repo"):
    if _p not in sys.path:
        sys.path.append(_p)

B, M, C = 8, 65536, 80
K = 32                  # anchors per partition-row per tile
P = 128
T = M // (P * K)        # 16 tiles
F = K * C               # 2560
NA = M // P             # 512 anchors per partition (all tiles)

_CACHED = {}


def _build_nc():
    import concourse.bacc as bacc
    import concourse.mybir as mybir
    import concourse.bass_isa as bass_isa
    from concourse.tile import TileContext

    AF = mybir.ActivationFunctionType
    ALU = mybir.AluOpType
    f32 = mybir.dt.float32
    bf16 = mybir.dt.bfloat16
    i32 = mybir.dt.int32
    u16 = mybir.dt.uint16
    u8 = mybir.dt.uint8

    # Force a single ACT table set: our three funcs (Exp, Ln, Square) all
    # live in natural_log_exp_and_others, but the greedy load inserter picks
    # the first containing set per func and ping-pongs (35 table loads,
    # ~2.7us each). Strip those funcs from every other set so one load wins.
    import concourse.bacc as _bacc_mod
    from concourse.hw_specs import get_activation_tables as _gat

    def _gat_oneset(arch):
        t = dict(_gat(arch))
        want = {AF.Exp, AF.Ln, AF.Square}
        if any(want <= fns for fns in t.values()):
            full = next(n for n, fns in t.items() if want <= fns)
            t = {n: (fns if n == full else fns - want) for n, fns in t.items()}
        return t

    _bacc_mod.get_activation_tables = _gat_oneset

    nc = bacc.Bacc("TRN2", target_bir_lowering=False, debug=False)
    x_ext = nc.declare_dram_parameter("x", [M, C], f32, isOutput=False)
    pb_ext = nc.declare_dram_parameter("pb", [M, 4], f32, isOutput=False)
    tb_ext = nc.declare_dram_parameter("tb", [M, 4], f32, isOutput=False)
    tgt_ext = nc.declare_dram_parameter("tgt", [M], i32, isOutput=False)
    msk_ext = nc.declare_dram_parameter("msk", [M], u8, isOutput=False)
    out_ext = nc.declare_dram_parameter("out4", [1, 4], f32, isOutput=True)

    xv = x_ext.ap().rearrange("(t p k) c -> t p (k c)", p=P, k=K)
    pav = lambda e: e.ap().rearrange("(t p k) -> t p k", p=P, k=K) \
        .transpose([1, 0, 2])   # noqa: E731
    pbv = lambda e: e.ap().rearrange("(t p k) c -> t p k c", p=P, k=K) \
        .transpose([1, 0, 2, 3])  # noqa: E731

    with TileContext(nc) as tc:
        with tc.tile_pool(name="pers", bufs=1) as pp, \
             tc.tile_pool(name="scratch", bufs=14) as sc, \
             tc.tile_pool(name="xpool", bufs=3) as xp, \
             tc.tile_pool(name="wpool", bufs=3) as wp, \
             tc.tile_pool(name="loop", bufs=2) as lp:
            # ---------------- persistent inputs ----------------
            tgt = pp.tile([P, NA], i32)
            nc.sync.dma_start(out=tgt[:, :], in_=pav(tgt_ext))
            msku = pp.tile([P, NA], u8)
            nc.sync.dma_start(out=msku[:, :], in_=pav(msk_ext))
            pb = pp.tile([P, NA * 4], f32)
            nc.sync.dma_start(out=pb[:, :], in_=pbv(pb_ext))
            tb = pp.tile([P, NA * 4], f32)
            nc.sync.dma_start(out=tb[:, :], in_=pbv(tb_ext))

            # ---------------- constants / masks ----------------
            mskf = pp.tile([P, NA], f32)
            nc.vector.tensor_copy(mskf[:, :], msku[:, :])
            tgtf = sc.tile([P, NA], f32, tag="s")
            nc.vector.tensor_copy(tgtf[:, :], tgt[:, :])
            fgm = pp.tile([P, NA], f32)      # 1.0 where tgt != 80
            nc.vector.tensor_scalar(out=fgm[:, :], in0=tgtf[:, :], scalar1=79.5,
                                    scalar2=None, op0=ALU.is_lt)
            vmf = pp.tile([P, NA], f32)      # valid * fg
            nc.gpsimd.tensor_tensor(out=vmf[:, :], in0=mskf[:, :], in1=fgm[:, :],
                                    op=ALU.mult)

            # select mask: selm[p, q] = (q == p % 16)
            q16 = pp.tile([P, 16], i32)
            nc.gpsimd.iota(q16[:, :], pattern=[[1, 16]], base=0,
                           channel_multiplier=0)
            pcol = pp.tile([P, 1], i32)
            nc.gpsimd.iota(pcol[:, :], pattern=[[0, 1]], base=0,
                           channel_multiplier=1)
            pmod = pp.tile([P, 1], i32)
            nc.vector.tensor_scalar(out=pmod[:, :], in0=pcol[:, :], scalar1=15,
                                    scalar2=None, op0=ALU.bitwise_and)
            pmodf = pp.tile([P, 1], f32)
            nc.vector.tensor_copy(pmodf[:, :], pmod[:, :])
            selm = pp.tile([P, 16], f32)
            nc.vector.tensor_scalar(out=selm[:, :], in0=q16[:, :],
                                    scalar1=pmodf[:, :], scalar2=None,
                                    op0=ALU.is_equal)

            # gather indices: idx[p, t*K+k] = k*C + min(tgt, 79)   (uint16)
            kvec = sc.tile([P, NA], i32, tag="s")
            nc.gpsimd.iota(kvec[:, :], pattern=[[0, T], [C, K]], base=0,
                           channel_multiplier=0)
            tcl = sc.tile([P, NA], i32, tag="s")
            nc.vector.tensor_scalar(out=tcl[:, :], in0=tgt[:, :], scalar1=79,
                                    scalar2=None, op0=ALU.min)
            idx = pp.tile([P, NA], u16)
            nc.vector.tensor_tensor(out=idx[:, :], in0=tcl[:, :], in1=kvec[:, :],
                                    op=ALU.add)

            accL = pp.tile([P, T], f32)      # per-tile label accum columns
            xg = pp.tile([P, NA], f32)       # gathered target logits

            # ------- GIoU emission as a generator (interleaved into loop) --
            pb3 = pb[:, :].rearrange("p (j c) -> p j c", c=4)
            tb3 = tb[:, :].rearrange("p (j c) -> p j c", c=4)
            cs = lambda a, i: a[:, :, i:i + 1]   # noqa: E731
            _gt = [0]

            def gtile():
                _gt[0] += 1
                return sc.tile([P, NA], f32, name=f"gt{_gt[0]}", tag="s")

            def gtt(o, a, b_, op, eng=None):
                (eng or nc.gpsimd).tensor_tensor(out=o, in0=a, in1=b_, op=op)

            v = lambda tl: tl[:, :].rearrange("p (j c) -> p j c", c=1)  # noqa: E731

            accG = pp.tile([P, 1], f32)
            accN = pp.tile([P, 1], f32)

            def giou_gen():
                ltx, lty, rbx, rby = gtile(), gtile(), gtile(), gtile()
                gtt(v(ltx), cs(pb3, 0), cs(tb3, 0), ALU.max, eng=nc.vector)
                gtt(v(lty), cs(pb3, 1), cs(tb3, 1), ALU.max, eng=nc.vector)
                yield
                gtt(v(rbx), cs(pb3, 2), cs(tb3, 2), ALU.min, eng=nc.vector)
                gtt(v(rby), cs(pb3, 3), cs(tb3, 3), ALU.min, eng=nc.vector)
                yield
                wx, wy = gtile(), gtile()
                gtt(wx[:, :], rbx[:, :], ltx[:, :], ALU.subtract)
                gtt(wy[:, :], rby[:, :], lty[:, :], ALU.subtract)
                yield
                nc.gpsimd.tensor_scalar(out=wx[:, :], in0=wx[:, :], scalar1=0.0,
                                        scalar2=None, op0=ALU.max)
                nc.gpsimd.tensor_scalar(out=wy[:, :], in0=wy[:, :], scalar1=0.0,
                                        scalar2=None, op0=ALU.max)
                yield
                inter = gtile()
                gtt(inter[:, :], wx[:, :], wy[:, :], ALU.mult)
                yield
                dx1, dy1, a1 = gtile(), gtile(), gtile()
                gtt(v(dx1), cs(pb3, 2), cs(pb3, 0), ALU.subtract, eng=nc.vector)
                gtt(v(dy1), cs(pb3, 3), cs(pb3, 1), ALU.subtract, eng=nc.vector)
                gtt(a1[:, :], dx1[:, :], dy1[:, :], ALU.mult)
                yield
                dx2, dy2, a2 = gtile(), gtile(), gtile()
                gtt(v(dx2), cs(tb3, 2), cs(tb3, 0), ALU.subtract, eng=nc.vector)
                gtt(v(dy2), cs(tb3, 3), cs(tb3, 1), ALU.subtract, eng=nc.vector)
                gtt(a2[:, :], dx2[:, :], dy2[:, :], ALU.mult)
                yield
                union = gtile()
                gtt(union[:, :], a1[:, :], a2[:, :], ALU.add)
                gtt(union[:, :], union[:, :], inter[:, :], ALU.subtract)
                yield
                cwx, cwy = gtile(), gtile()
                gtt(v(cwx), cs(pb3, 0), cs(tb3, 0), ALU.min, eng=nc.vector)
                gtt(v(cwy), cs(pb3, 2), cs(tb3, 2), ALU.max, eng=nc.vector)
                gtt(cwx[:, :], cwy[:, :], cwx[:, :], ALU.subtract)  # width
                yield
                chy, chy2 = gtile(), gtile()
                gtt(v(chy), cs(pb3, 1), cs(tb3, 1), ALU.min, eng=nc.vector)
                gtt(v(chy2), cs(pb3, 3), cs(tb3, 3), ALU.max, eng=nc.vector)
                gtt(chy[:, :], chy2[:, :], chy[:, :], ALU.subtract)  # height
                yield
                areac = gtile()
                gtt(areac[:, :], cwx[:, :], chy[:, :], ALU.mult)
                yield
                ru = gtile()
                nc.vector.reciprocal(out=ru[:, :], in_=union[:, :])
                iou = gtile()
                nc.vector.tensor_tensor(out=iou[:, :], in0=inter[:, :],
                                        in1=ru[:, :], op=ALU.mult)
                yield
                amu = gtile()
                gtt(amu[:, :], areac[:, :], union[:, :], ALU.subtract)
                rc = gtile()
                nc.vector.reciprocal(out=rc[:, :], in_=areac[:, :])
                yield
                pen = gtile()
                nc.vector.tensor_tensor(out=pen[:, :], in0=amu[:, :],
                                        in1=rc[:, :], op=ALU.mult)
                giou = gtile()
                nc.vector.tensor_tensor(out=giou[:, :], in0=iou[:, :],
                                        in1=pen[:, :], op=ALU.subtract)
                yield
                # sum (1 - giou) * fg  =  sum(fg) + sum(-giou * fg)
                gneg = gtile()
                nc.vector.scalar_tensor_tensor(out=gneg[:, :], in0=giou[:, :],
                                               scalar=-1.0, in1=fgm[:, :],
                                               op0=ALU.mult, op1=ALU.mult,
                                               accum_out=accG[:, :])
                nc.vector.tensor_reduce(out=accN[:, :], in_=fgm[:, :],
                                        axis=mybir.AxisListType.X, op=ALU.add)
                yield

            gio = giou_gen()

            # ------- target-class correction over a column range ----------
            accC = pp.tile([P, 2], f32)

            def corr_emit(half):
                j0, j1 = (0, NA // 2) if half == 0 else (NA // 2, NA)
                xs = xg[:, j0:j1]
                n = j1 - j0
                e2g = sc.tile([P, n], f32, name=f"e2g{half}", tag="s")
                nc.scalar.activation(out=e2g[:, :], in_=xs, func=AF.Exp)
                wg = sc.tile([P, n], f32, name=f"wg{half}", tag="s")
                nc.scalar.activation(out=wg[:, :], in_=e2g[:, :], func=AF.Ln,
                                     bias=1.0)
                ug = sc.tile([P, n], f32, name=f"ug{half}", tag="s")
                nc.scalar.activation(out=ug[:, :], in_=wg[:, :], func=AF.Exp,
                                     scale=-1.0)
                sqT = sc.tile([P, n], f32, name=f"sqT{half}", tag="s")
                nc.scalar.activation(out=sqT[:, :], in_=ug[:, :], func=AF.Square)
                sqB = sc.tile([P, n], f32, name=f"sqB{half}", tag="s")
                nc.scalar.activation(out=sqB[:, :], in_=ug[:, :], func=AF.Square,
                                     scale=-1.0, bias=1.0)
                Lg = sc.tile([P, n], f32, name=f"Lg{half}", tag="s")
                nc.vector.tensor_tensor(out=Lg[:, :], in0=wg[:, :], in1=xs,
                                        op=ALU.subtract)
                T1 = sc.tile([P, n], f32, name=f"T1_{half}", tag="s")
                nc.vector.scalar_tensor_tensor(out=T1[:, :], in0=Lg[:, :],
                                               scalar=0.25, in1=sqT[:, :],
                                               op0=ALU.mult, op1=ALU.mult)
                T0 = sc.tile([P, n], f32, name=f"T0_{half}", tag="s")
                nc.vector.scalar_tensor_tensor(out=T0[:, :], in0=wg[:, :],
                                               scalar=0.75, in1=sqB[:, :],
                                               op0=ALU.mult, op1=ALU.mult)
                corr = sc.tile([P, n], f32, name=f"corr{half}", tag="s")
                nc.vector.tensor_tensor(out=corr[:, :], in0=T1[:, :],
                                        in1=T0[:, :], op=ALU.subtract)
                corrm = sc.tile([P, n], f32, name=f"corrm{half}", tag="s")
                nc.vector.scalar_tensor_tensor(out=corrm[:, :], in0=corr[:, :],
                                               scalar=1.0, in1=vmf[:, j0:j1],
                                               op0=ALU.mult, op1=ALU.mult,
                                               accum_out=accC[:, half:half + 1])

            # ---------------- big loop over pred_cls tiles ----------------
            for t in range(T):
                x_t = xp.tile([P, F], f32, tag="x")
                nc.sync.dma_start(out=x_t[:, :], in_=xv[t])
                # gather target logits (16x group gather + diagonal select)
                g16 = lp.tile([P, K * 16], f32, tag="g16")
                nc.gpsimd.indirect_copy(g16[:, :], x_t[:, :],
                                        idx[:, t * K:(t + 1) * K],
                                        i_know_ap_gather_is_preferred=True)
                gm = lp.tile([P, K * 16], f32, tag="gm")
                g3 = g16[:, :].rearrange("p (k q) -> p k q", q=16)
                gm3 = gm[:, :].rearrange("p (k q) -> p k q", q=16)
                selm_b = selm[:, :].unsqueeze(1).broadcast_to([P, K, 16])
                nc.gpsimd.tensor_tensor(out=gm3, in0=g3, in1=selm_b,
                                        op=ALU.mult)
                nc.vector.tensor_reduce(out=xg[:, t * K:(t + 1) * K], in_=gm3,
                                        axis=mybir.AxisListType.X, op=ALU.add)
                e2 = lp.tile([P, F], f32, tag="e2dd")
                nc.scalar.activation(out=e2[:, :], in_=x_t[:, :], func=AF.Exp)
                w = wp.tile([P, F], bf16, tag="w")
                nc.scalar.activation(out=w[:, :], in_=e2[:, :], func=AF.Ln,
                                     bias=1.0)
                dd = lp.tile([P, F], f32, tag="e2dd")
                dd_eng = nc.vector if t % 2 == 0 else nc.gpsimd
                dd_eng.tensor_tensor(out=dd[:, :], in0=x_t[:, :], in1=w[:, :],
                                     op=ALU.subtract)
                sq = lp.tile([P, F], bf16, tag="sq")
                nc.scalar.activation(out=sq[:, :], in_=dd[:, :], func=AF.Exp,
                                     scale=2.0)
                h = lp.tile([P, F], bf16, tag="h")
                nc.vector.scalar_tensor_tensor(out=h[:, :], in0=w[:, :],
                                               scalar=0.75, in1=sq[:, :],
                                               op0=ALU.mult, op1=ALU.mult)
                # masked accumulate (in-place): accL[:, t] = sum(h * valid)
                m_b = mskf[:, t * K:(t + 1) * K].unsqueeze(2) \
                    .broadcast_to([P, K, C])
                nc.vector.scalar_tensor_tensor(
                    out=h[:, :].rearrange("p (k c) -> p k c", c=C),
                    in0=h[:, :].rearrange("p (k c) -> p k c", c=C),
                    scalar=1.0, in1=m_b, op0=ALU.mult, op1=ALU.mult,
                    accum_out=accL[:, t:t + 1])
                # interleave a slice of GIoU work
                next(gio, None)
                if t == 8:
                    corr_emit(0)

            for _ in range(20):
                next(gio, None)
            corr_emit(1)

            # ---------------- final combine + partition reduce ------------
            accLs = pp.tile([P, 1], f32)
            nc.vector.tensor_reduce(out=accLs[:, :], in_=accL[:, :],
                                    axis=mybir.AxisListType.X, op=ALU.add)
            accCs = pp.tile([P, 1], f32)
            nc.vector.tensor_reduce(out=accCs[:, :], in_=accC[:, :],
                                    axis=mybir.AxisListType.X, op=ALU.add)
            slab = pp.tile([P, 1], f32)
            nc.vector.tensor_tensor(out=slab[:, :], in0=accLs[:, :],
                                    in1=accCs[:, :], op=ALU.add)
            sg = pp.tile([P, 1], f32)
            nc.vector.tensor_tensor(out=sg[:, :], in0=accN[:, :],
                                    in1=accG[:, :], op=ALU.add)
            pack = pp.tile([P, 4], f32)
            nc.vector.memset(pack[:, :], 0.0)
            nc.vector.tensor_copy(pack[:, 0:1], slab[:, :])
            nc.vector.tensor_copy(pack[:, 1:2], sg[:, :])
            nc.vector.tensor_copy(pack[:, 2:3], accN[:, :])
            red = pp.tile([P, 4], f32)
            nc.gpsimd.partition_all_reduce(red[:, :], pack[:, :], channels=P,
                                           reduce_op=bass_isa.ReduceOp.add)
            nc.sync.dma_start(out=out_ext[:, :], in_=red[0:1, :])

    nc.finalize()
    return nc


def _get_nc():
    if "nc" not in _CACHED:
        _CACHED["nc"] = _build_nc()
    return _CACHED["nc"]


def kernel(pred_cls, pred_box, tgt_classes, tgt_boxes, mask, _trace=False):
    from concourse.bass_utils import run_bass_kernel_spmd

    nc = _get_nc()
    in_maps = []
    for b in range(B):
        in_maps.append({
            "x": np.ascontiguousarray(pred_cls[b], dtype=np.float32)
                 .reshape(M, C),
            "pb": np.ascontiguousarray(pred_box[b], dtype=np.float32)
                  .reshape(M, 4),
            "tb": np.ascontiguousarray(tgt_boxes[b], dtype=np.float32)
                  .reshape(M, 4),
            "tgt": np.ascontiguousarray(tgt_classes[b]).astype(np.int32)
                   .reshape(M),
            "msk": np.ascontiguousarray(mask[b]).astype(np.uint8).reshape(M),
        })
    res = run_bass_kernel_spmd(nc, in_maps, list(range(B)), trace=_trace)
    sl = sg = nf = 0.0
    for r in res.results:
        o = r["out4"][0]
        sl += float(o[0])
        sg += float(o[1])
        nf += float(o[2])
    num_fg = max(nf, 1.0)
    ll = np.float32(np.float32(sl) / np.float32(num_fg))
    lb = np.float32(np.float32(sg) / np.float32(num_fg))
    losses = np.float32(ll + lb)
    if _trace:
        return (ll, lb, losses), res
    return (ll, lb, losses)
